# revision 25
# baseline (speedup 1.0000x reference)
"""BinaryTreeLSTM forest kernel for Trainium2 (Bass/Tile), 8-core SPMD.

Problem (hardcoded): B=128 complete binary trees, L=256 leaves each,
H=512, VOCAB=32000.  Leaves: h = emb[word_idx], c = 0.5.  8 level-
synchronous combine steps with a 2x(5H x H) gate GEMM per node.

Sharding: data-parallel across trees -- 16 trees per NeuronCore, weights
and embedding table replicated.  No collectives.

Device layout: h/c live as [H-chunk-on-128-partitions x nodes-on-free]
("H-major, chunk-major") so the gate GEMMs contract H on the partition
axis with fp32r (full-rate fp32) matmuls; weights are host-permuted
g-major so each H-chunk's five gates are contiguous.  Leaf embeddings
are gathered row-major with indirect DMA (Pool queue only) and
transposed on the PE.  Levels 1-2 round-trip h/c through tracked DRAM
tiles (stores+loads share the SP queue, in dependency order); levels
3-8 stay SBUF-resident.  fp32r's 4x slowdown below 256 moving columns
is dodged at S=128 by streaming the rhs twice through a 0-stride AP
dim.  Internal h for every level lands in DRAM level-major; the host
applies the static post-order permutation (including the reference's
duplicated-first-node quirk) while unsharding.  c stays full fp32
end-to-end; only h passes through fp32r rounding (measured 5.5e-4
relative absmax vs the fp32 reference).
"""

import numpy as np
from contextlib import ExitStack

# ---------------------------------------------------------------- constants
B, L, H, VOCAB = 128, 256, 512, 32000
DEPTH = 8
NCORES = 8
NT = B // NCORES          # trees per core = 16
P = 128                   # SBUF partitions
KC = H // P               # 4 H-chunks
MC = 5 * H // P           # 20 gate chunks
G5 = 5 * H                # 2560
S_MAX = 256               # parent nodes per compute tile

# per-tree internal-node level offsets (level-major), levels 1..8
OFF = [0]
_s = 0
for _k in range(1, DEPTH + 1):
    OFF.append(_s)
    _s += L >> _k
# OFF[k] for k in 1..8 = [0,128,192,224,240,248,252,254]


def _post_order():
    acc = []

    def rec(k, j):
        if k == 0:
            return
        rec(k - 1, 2 * j)
        rec(k - 1, 2 * j + 1)
        acc.append(OFF[k] + j)

    rec(DEPTH, 0)
    return np.asarray(acc, dtype=np.int64)


POST = _post_order()      # (255,)


def _lvl_meta(nt):
    """Per-core level sizes and column bases for nt trees."""
    lvl_n = [nt * (128 >> k) for k in range(DEPTH)]
    base = [0]
    for n in lvl_n:
        base.append(base[-1] + n)
    return lvl_n, base      # base[DEPTH] == nt*255


# ---------------------------------------------------------------- device IR
_PROG_CACHE = {}


def _build_program(nt=NT, reps=1):
    if (nt, reps) in _PROG_CACHE:
        return _PROG_CACHE[(nt, reps)]

    import concourse.mybir as mybir
    import concourse.tile as tile
    from concourse import bacc
    from concourse import bass as bass_mod
    from concourse.bass import IndirectOffsetOnAxis
    from concourse.masks import make_identity

    dt = mybir.dt
    AF = mybir.ActivationFunctionType
    OP = mybir.AluOpType

    lvl_n, lvl_base = _lvl_meta(nt)
    total = lvl_base[DEPTH]          # nt*255
    scr_cols = lvl_base[2]           # levels 1-2 round-trip via DRAM

    nc = bacc.Bacc("TRN2", target_bir_lowering=False, debug=False)

    idx_d = nc.dram_tensor("idx", [P, 2 * nt], dt.int32, kind="ExternalInput")
    emb_d = nc.dram_tensor("emb", [VOCAB, H], dt.float32, kind="ExternalInput")
    wlt_d = nc.dram_tensor("wlt", [H, G5], dt.float32r, kind="ExternalInput")
    wrt_d = nc.dram_tensor("wrt", [H, G5], dt.float32r, kind="ExternalInput")
    bias_d = nc.dram_tensor("bias", [P, MC], dt.float32, kind="ExternalInput")
    hs_d = nc.dram_tensor("hs", [H, total], dt.float32r, kind="ExternalOutput")
    croot_d = nc.dram_tensor("c_root", [H, nt], dt.float32, kind="ExternalOutput")

    def d3(ap2d):
        # (512, n) DRAM view -> (128, KC, n): row = c*128 + p
        return ap2d.rearrange("(c p) n -> p c n", p=P)

    def s3(t):
        # (128, KC*n) SBUF tile -> (128, KC, n)
        return t.rearrange("p (c n) -> p c n", c=KC)

    with tile.TileContext(nc) as tc, ExitStack() as ctx:
        wpool = ctx.enter_context(tc.tile_pool(name="wpool", bufs=1))
        dpool = ctx.enter_context(tc.tile_pool(name="dpool", bufs=1, space="DRAM"))
        leafp = ctx.enter_context(tc.tile_pool(name="leafp", bufs=3))
        hcin = ctx.enter_context(tc.tile_pool(name="hcin", bufs=2))
        sigp = ctx.enter_context(tc.tile_pool(name="sigp", bufs=1))
        tmpp = ctx.enter_context(tc.tile_pool(name="tmpp", bufs=2))
        outp = ctx.enter_context(tc.tile_pool(name="outp", bufs=1))
        statp = ctx.enter_context(tc.tile_pool(name="statp", bufs=2))
        psg = ctx.enter_context(tc.tile_pool(name="psg", bufs=4, space="PSUM"))
        pst = ctx.enter_context(tc.tile_pool(name="pst", bufs=4, space="PSUM"))

        # --- persistent inputs; idx/bias first so the leaf pipeline can
        # start while the 10.5MB of weights stream in, weights in
        # m-ascending interleaved chunks so early gate MMs unblock first.
        idx_sb = wpool.tile([P, 2 * nt], dt.int32, name="idx_sb")
        nc.sync.dma_start(out=idx_sb[:], in_=idx_d.ap()[:])
        bias_sb = wpool.tile([P, MC], dt.float32, name="bias_sb")
        nc.sync.dma_start(out=bias_sb[:], in_=bias_d.ap()[:])
        ident = wpool.tile([P, P], dt.float32, name="ident")
        make_identity(nc, ident[:])

        wl_sb = [wpool.tile([P, G5], dt.float32r, name=f"wl{kk}")
                 for kk in range(KC)]
        wr_sb = [wpool.tile([P, G5], dt.float32r, name=f"wr{kk}")
                 for kk in range(KC)]
        WCH = G5 // 4      # 640-column (5 gate-chunk) load granularity
        for mq in range(4):
            cs_ = slice(mq * WCH, (mq + 1) * WCH)
            for kk in range(KC):
                nc.sync.dma_start(out=wl_sb[kk][:, cs_],
                                  in_=wlt_d.ap()[kk * P:(kk + 1) * P, cs_])
                nc.sync.dma_start(out=wr_sb[kk][:, cs_],
                                  in_=wrt_d.ap()[kk * P:(kk + 1) * P, cs_])

        h_scr = dpool.tile([H, scr_cols], dt.float32r, name="h_scr")
        c_scr = dpool.tile([H, scr_cols], dt.float32, name="c_scr")

        RES_FROM = 2          # outputs of k >= RES_FROM stay SBUF-resident

        for _rep in range(reps):
          h_state = c_state = None          # previous level's resident tiles
          for k in range(DEPTH):
            n_lvl = lvl_n[k]
            S = min(512 if k == 0 else S_MAX, n_lvl)
            W = 2 * S
            resident = k >= RES_FROM
            if resident:
                h_lvl = statp.tile([P, KC * n_lvl], dt.float32r,
                                   tag="state_h", name="h_lvl")
                c_lvl = statp.tile([P, KC * n_lvl], dt.float32,
                                   tag="state_c", name="c_lvl")
            h_prev, c_prev = h_state, c_state
            for it in range(n_lvl // S):
                a = it * S
                col0 = lvl_base[k] + a

                if k == 0:
                    hL = hcin.tile([P, KC * S], dt.float32r, tag="h_in", name="hL")
                    hR = hcin.tile([P, KC * S], dt.float32r, tag="c_in", name="hR")
                    for tt in range(S // 128):
                        t = a // 128 + tt
                        for par, dst in ((0, hL), (1, hR)):
                            stage = leafp.tile([P, H], dt.float32, tag="stage",
                                               name="stage")
                            nc.gpsimd.indirect_dma_start(
                                out=stage[:],
                                out_offset=None,
                                in_=emb_d.ap(),
                                in_offset=IndirectOffsetOnAxis(
                                    ap=idx_sb[:, 2 * t + par:2 * t + par + 1],
                                    axis=0,
                                ),
                            )
                            for c in range(KC):
                                ptr = pst.tile([P, P], dt.float32, tag="tr",
                                               name="ptr")
                                nc.tensor.transpose(
                                    ptr[:], stage[:, c * P:(c + 1) * P],
                                    ident[:])
                                o0 = c * S + tt * 128
                                nc.vector.tensor_copy(dst[:, o0:o0 + 128], ptr[:])

                    def rhs(kk, side, _hL=hL, _hR=hR, _S=S):
                        src = _hL if side == 0 else _hR
                        return src[:, kk * _S:(kk + 1) * _S]

                    cin = None
                elif k <= RES_FROM:
                    # children streamed back from DRAM scratch
                    c0 = lvl_base[k - 1] + 2 * a
                    h_in = hcin.tile([P, KC * W], dt.float32r, tag="h_in",
                                     name="h_in")
                    c_in = hcin.tile([P, KC * W], dt.float32, tag="c_in",
                                     name="c_in")
                    for w0 in range(0, W, 256):
                        hw_ = min(256, W - w0)
                        nc.sync.dma_start(
                            out=s3(h_in)[:, :, w0:w0 + hw_],
                            in_=d3(h_scr[:, c0:c0 + W])[:, :, w0:w0 + hw_])
                        nc.sync.dma_start(
                            out=s3(c_in)[:, :, w0:w0 + hw_],
                            in_=d3(c_scr[:, c0:c0 + W])[:, :, w0:w0 + hw_])

                    def rhs(kk, side, _h=h_in, _W=W):
                        b0 = kk * _W
                        return _h[:, b0 + side:b0 + _W:2]

                    def cin(g, side, _c=c_in, _W=W):
                        b0 = g * _W
                        return _c[:, b0 + side:b0 + _W:2]
                else:
                    # children live in the previous level's resident tiles
                    n_prev = 2 * n_lvl

                    def rhs(kk, side, _h=h_prev, _np=n_prev, _a=2 * a, _W=W):
                        b0 = kk * _np + _a
                        return _h[:, b0 + side:b0 + _W:2]

                    def cin(g, side, _c=c_prev, _np=n_prev, _a=2 * a, _W=W):
                        b0 = g * _np + _a
                        return _c[:, b0 + side:b0 + _W:2]

                if resident:
                    h_out = h_lvl[:, :]
                    c_out = c_lvl[:, :]
                    ho_sl = lambda g, _a=a, _n=n_lvl, _S=S: \
                        h_out[:, g * _n + _a:g * _n + _a + _S]
                    co_sl = lambda g, _a=a, _n=n_lvl, _S=S: \
                        c_out[:, g * _n + _a:g * _n + _a + _S]
                else:
                    h_out = outp.tile([P, KC * S], dt.float32r, tag="h_out",
                                      name="h_out")
                    c_out = outp.tile([P, KC * S], dt.float32, tag="c_out",
                                      name="c_out")
                    ho_sl = lambda g, _S=S: h_out[:, g * _S:(g + 1) * _S]
                    co_sl = lambda g, _S=S: c_out[:, g * _S:(g + 1) * _S]

                # fp32r matmuls drop to 1/4 rate below 256 moving columns;
                # at S=128 recover full rate by streaming the rhs twice via
                # a 0-stride AP dim (psum columns S..2S are discarded dups)
                pad = 2 if (S == 128 and k > 0) else 1

                for g in range(KC):
                    sigs = {}
                    # weights are host-permuted g-major: the 5 gate chunks
                    # for H-chunk g sit at m = 5g..5g+4 (i, fl, fr, cc, o)
                    for role, m in (("i", 5 * g), ("fl", 5 * g + 1),
                                    ("fr", 5 * g + 2), ("cc", 5 * g + 3),
                                    ("o", 5 * g + 4)):
                        ps = psg.tile([P, S * pad], dt.float32, tag="gate",
                                      name="ps")
                        first = True
                        for side in (0, 1):
                            wsb = wl_sb if side == 0 else wr_sb
                            for kk in range(KC):
                                r_ap = rhs(kk, side)
                                if pad == 2:
                                    r_ap = bass_mod.AP(
                                        r_ap.tensor, r_ap.offset,
                                        [list(r_ap.ap[0]), [0, 2],
                                         list(r_ap.ap[-1])])
                                nc.tensor.matmul(
                                    ps[:],
                                    lhsT=wsb[kk][:, m * P:(m + 1) * P],
                                    rhs=r_ap,
                                    start=first,
                                    stop=(side == 1 and kk == KC - 1),
                                )
                                first = False
                        sg = sigp.tile([P, S], dt.float32, tag=f"sig_{role}",
                                       name=f"sig_{role}")
                        nc.scalar.activation(
                            sg[:], ps[:, :S],
                            AF.Tanh if role == "cc" else AF.Sigmoid,
                            bias=bias_sb[:, m:m + 1],
                        )
                        sigs[role] = sg

                    cq = co_sl(g)
                    ta = tmpp.tile([P, S], dt.float32, tag="ta", name="ta")
                    nc.vector.tensor_mul(ta[:], sigs["i"][:], sigs["cc"][:])
                    if k == 0:
                        u = tmpp.tile([P, S], dt.float32, tag="u", name="u")
                        nc.vector.tensor_add(u[:], sigs["fl"][:], sigs["fr"][:])
                        nc.vector.scalar_tensor_tensor(
                            out=cq, in0=u[:], scalar=0.5, in1=ta[:],
                            op0=OP.mult, op1=OP.add)
                    else:
                        u = tmpp.tile([P, S], dt.float32, tag="u", name="u")
                        nc.vector.tensor_mul(u[:], sigs["fl"][:], cin(g, 0))
                        v = tmpp.tile([P, S], dt.float32, tag="v", name="v")
                        nc.vector.tensor_mul(v[:], sigs["fr"][:], cin(g, 1))
                        w2 = tmpp.tile([P, S], dt.float32, tag="w2", name="w2")
                        nc.vector.tensor_add(w2[:], ta[:], u[:])
                        nc.vector.tensor_add(cq, w2[:], v[:])
                    tcq = tmpp.tile([P, S], dt.float32, tag="tcq", name="tcq")
                    nc.scalar.activation(tcq[:], cq, AF.Tanh)
                    nc.vector.tensor_mul(ho_sl(g), sigs["o"][:], tcq[:])

                if not resident:
                    # per-tile stores of h (output + scratch) and c (scratch)
                    nc.sync.dma_start(out=d3(hs_d.ap()[:, col0:col0 + S]),
                                      in_=s3(h_out))
                    nc.sync.dma_start(out=d3(h_scr[:, col0:col0 + S]),
                                      in_=s3(h_out))
                    nc.sync.dma_start(out=d3(c_scr[:, col0:col0 + S]),
                                      in_=s3(c_out))

            if resident:
                lb = lvl_base[k]
                nc.sync.dma_start(
                    out=d3(hs_d.ap()[:, lb:lb + n_lvl]), in_=s3(h_lvl[:, :]))
                if k == DEPTH - 1:
                    nc.sync.dma_start(out=d3(croot_d.ap()),
                                      in_=s3(c_lvl[:, :]))
                h_state, c_state = h_lvl, c_lvl

    nc.compile()
    _PROG_CACHE[(nt, reps)] = nc
    return nc


# ---------------------------------------------------------------- host side
def _prep_inputs(word_idx, emb, Wl, Wr, b, ncores=NCORES, nt=NT):
    """Build per-core input maps."""
    wi = np.asarray(word_idx).astype(np.int32)          # (B, L)
    # idx[core][p, 2*t+par] = word_idx[core*nt + t, 2*p + par]
    idx = wi.reshape(ncores, nt, 128, 2).transpose(0, 2, 1, 3) \
        .reshape(ncores, 128, 2 * nt)
    idx = np.ascontiguousarray(idx)

    embf = np.ascontiguousarray(np.asarray(emb, dtype=np.float32))

    # permute gate rows g-major: new chunk 5g+pos = old chunk r*KC+g where
    # pos->(i,fl,fr,cc,o) maps to old gate r in (0,1,2,4,3)
    perm = np.empty(MC, dtype=np.int64)
    for g in range(KC):
        for pos, r in enumerate((0, 1, 2, 4, 3)):
            perm[5 * g + pos] = r * KC + g
    row_perm = (perm[:, None] * P + np.arange(P)[None, :]).reshape(-1)

    wlt = np.ascontiguousarray(Wl[row_perm].T.astype(np.float32))   # (H, 5H)
    wrt = np.ascontiguousarray(Wr[row_perm].T.astype(np.float32))
    bias_t = np.ascontiguousarray(
        b[row_perm].astype(np.float32).reshape(MC, P).T)            # (128, 20)

    return [
        {"idx": idx[c], "emb": embf, "wlt": wlt, "wrt": wrt, "bias": bias_t}
        for c in range(ncores)
    ]


def _assemble(results, ncores=NCORES, nt=NT):
    """Gather per-core (hs, c_root) into the reference output pytree."""
    lvl_n, lvl_base = _lvl_meta(nt)
    H_int = np.empty((ncores * nt, L - 1, H), dtype=np.float32)
    fh = np.empty((1, ncores * nt, H), dtype=np.float32)
    fc = np.empty((1, ncores * nt, H), dtype=np.float32)
    for c in range(ncores):
        hs = np.asarray(results[c]["hs"])            # (512, nt*255)
        croot = np.asarray(results[c]["c_root"])     # (512, nt)
        for k in range(DEPTH):
            nk = 128 >> k
            blk = hs[:, lvl_base[k]:lvl_base[k + 1]].reshape(H, nt, nk)
            H_int[c * nt:(c + 1) * nt, OFF[k + 1]:OFF[k + 1] + nk, :] = \
                blk.transpose(1, 2, 0)
        fh[0, c * nt:(c + 1) * nt, :] = \
            hs[:, lvl_base[DEPTH - 1]:lvl_base[DEPTH]].T
        fc[0, c * nt:(c + 1) * nt, :] = croot.T
    out = np.concatenate([H_int[:, POST[:1]], H_int[:, POST]], axis=1)
    return out, (fh, fc)


def kernel(word_idx, emb, U_i_l, U_i_r, b_i, U_fl_l, U_fl_r, b_fl,
           U_fr_l, U_fr_r, b_fr, U_o_l, U_o_r, b_o, U_c_l, U_c_r, b_c):
    from concourse.bass_utils import run_bass_kernel_spmd

    Wl = np.concatenate([np.asarray(x, np.float32)
                         for x in (U_i_l, U_fl_l, U_fr_l, U_o_l, U_c_l)], axis=0)
    Wr = np.concatenate([np.asarray(x, np.float32)
                         for x in (U_i_r, U_fl_r, U_fr_r, U_o_r, U_c_r)], axis=0)
    b = np.concatenate([np.asarray(x, np.float32)
                        for x in (b_i, b_fl, b_fr, b_o, b_c)], axis=0)

    in_maps = _prep_inputs(word_idx, emb, Wl, Wr, b)
    nc = _build_program(NT)
    res = run_bass_kernel_spmd(nc, in_maps, list(range(NCORES)))
    return _assemble(res.results)


# revision 27
# speedup vs baseline: 1.0760x; 1.0760x over previous
"""BinaryTreeLSTM forest kernel for Trainium2 (Bass/Tile), 8-core SPMD.

Problem (hardcoded): B=128 complete binary trees, L=256 leaves each,
H=512, VOCAB=32000.  Leaves: h = emb[word_idx], c = 0.5.  8 level-
synchronous combine steps with a 2x(5H x H) gate GEMM per node.

Sharding: data-parallel across trees -- 16 trees per NeuronCore, weights
and embedding table replicated.  No collectives.

Device layout: h/c live as [H-chunk-on-128-partitions x nodes-on-free]
("H-major, chunk-major") so the gate GEMMs contract H on the partition
axis with fp32r (full-rate fp32) matmuls; weights are host-permuted
g-major so each H-chunk's five gates are contiguous.  Leaf embeddings
are gathered row-major with indirect DMA (Pool queue only) and
transposed on the PE.  Levels 1-2 round-trip h/c through tracked DRAM
tiles (stores+loads share the SP queue, in dependency order); levels
3-8 stay SBUF-resident.  fp32r's 4x slowdown below 256 moving columns
is dodged at S=128 by streaming the rhs twice through a 0-stride AP
dim.  Internal h for every level lands in DRAM level-major; the host
applies the static post-order permutation (including the reference's
duplicated-first-node quirk) while unsharding.  c stays full fp32
end-to-end; only h passes through fp32r rounding (measured 5.5e-4
relative absmax vs the fp32 reference).
"""

import numpy as np
from contextlib import ExitStack

# ---------------------------------------------------------------- constants
B, L, H, VOCAB = 128, 256, 512, 32000
DEPTH = 8
NCORES = 8
NT = B // NCORES          # trees per core = 16
P = 128                   # SBUF partitions
KC = H // P               # 4 H-chunks
MC = 5 * H // P           # 20 gate chunks
G5 = 5 * H                # 2560
S_MAX = 256               # parent nodes per compute tile

# per-tree internal-node level offsets (level-major), levels 1..8
OFF = [0]
_s = 0
for _k in range(1, DEPTH + 1):
    OFF.append(_s)
    _s += L >> _k
# OFF[k] for k in 1..8 = [0,128,192,224,240,248,252,254]


def _post_order():
    acc = []

    def rec(k, j):
        if k == 0:
            return
        rec(k - 1, 2 * j)
        rec(k - 1, 2 * j + 1)
        acc.append(OFF[k] + j)

    rec(DEPTH, 0)
    return np.asarray(acc, dtype=np.int64)


POST = _post_order()      # (255,)


def _lvl_meta(nt):
    """Per-core level sizes and column bases for nt trees."""
    lvl_n = [nt * (128 >> k) for k in range(DEPTH)]
    base = [0]
    for n in lvl_n:
        base.append(base[-1] + n)
    return lvl_n, base      # base[DEPTH] == nt*255


# ---------------------------------------------------------------- device IR
_PROG_CACHE = {}


def _build_program(nt=NT, reps=1):
    if (nt, reps) in _PROG_CACHE:
        return _PROG_CACHE[(nt, reps)]

    import concourse.mybir as mybir
    import concourse.tile as tile
    from concourse import bacc
    from concourse import bass as bass_mod
    from concourse.bass import IndirectOffsetOnAxis
    from concourse.masks import make_identity

    dt = mybir.dt
    AF = mybir.ActivationFunctionType
    OP = mybir.AluOpType

    lvl_n, lvl_base = _lvl_meta(nt)
    total = lvl_base[DEPTH]          # nt*255
    scr_cols = lvl_base[2]           # levels 1-2 round-trip via DRAM

    nc = bacc.Bacc("TRN2", target_bir_lowering=False, debug=False)

    idx_d = nc.dram_tensor("idx", [P, 2 * nt], dt.int32, kind="ExternalInput")
    emb_d = nc.dram_tensor("emb", [VOCAB, H], dt.float32, kind="ExternalInput")
    wlt_d = nc.dram_tensor("wlt", [H, G5], dt.float32r, kind="ExternalInput")
    wrt_d = nc.dram_tensor("wrt", [H, G5], dt.float32r, kind="ExternalInput")
    bias_d = nc.dram_tensor("bias", [P, MC], dt.float32, kind="ExternalInput")
    biasr_d = nc.dram_tensor("bias_r", [1, G5], dt.float32r, kind="ExternalInput")
    hs_d = nc.dram_tensor("hs", [H, total], dt.float32r, kind="ExternalOutput")
    croot_d = nc.dram_tensor("c_root", [H, nt], dt.float32, kind="ExternalOutput")

    def d3(ap2d):
        # (512, n) DRAM view -> (128, KC, n): row = c*128 + p
        return ap2d.rearrange("(c p) n -> p c n", p=P)

    def s3(t):
        # (128, KC*n) SBUF tile -> (128, KC, n)
        return t.rearrange("p (c n) -> p c n", c=KC)

    with tile.TileContext(nc) as tc, ExitStack() as ctx:
        wpool = ctx.enter_context(tc.tile_pool(name="wpool", bufs=1))
        dpool = ctx.enter_context(tc.tile_pool(name="dpool", bufs=1, space="DRAM"))
        leafp = ctx.enter_context(tc.tile_pool(name="leafp", bufs=3))
        hcin = ctx.enter_context(tc.tile_pool(name="hcin", bufs=2))
        sigp = ctx.enter_context(tc.tile_pool(name="sigp", bufs=1))
        tmpp = ctx.enter_context(tc.tile_pool(name="tmpp", bufs=2))
        outp = ctx.enter_context(tc.tile_pool(name="outp", bufs=1))
        statp = ctx.enter_context(tc.tile_pool(name="statp", bufs=2))
        psg = ctx.enter_context(tc.tile_pool(name="psg", bufs=4, space="PSUM"))
        pst = ctx.enter_context(tc.tile_pool(name="pst", bufs=4, space="PSUM"))

        # --- persistent inputs; idx/bias first so the leaf pipeline can
        # start while the 10.5MB of weights stream in, weights in
        # m-ascending interleaved chunks so early gate MMs unblock first.
        idx_sb = wpool.tile([P, 2 * nt], dt.int32, name="idx_sb")
        nc.sync.dma_start(out=idx_sb[:], in_=idx_d.ap()[:])
        bias_sb = wpool.tile([P, MC], dt.float32, name="bias_sb")
        nc.sync.dma_start(out=bias_sb[:], in_=bias_d.ap()[:])
        biasr_sb = wpool.tile([1, G5], dt.float32r, name="biasr_sb")
        nc.sync.dma_start(out=biasr_sb[:], in_=biasr_d.ap()[:])
        ones_f = wpool.tile([1, P], dt.float32, name="ones_f")
        nc.gpsimd.memset(ones_f[:], 1.0)
        ones_r = wpool.tile([1, P], dt.float32r, name="ones_r")
        nc.vector.tensor_copy(ones_r[:], ones_f[:])
        ident = wpool.tile([P, P], dt.float32, name="ident")
        make_identity(nc, ident[:])

        wl_sb = [wpool.tile([P, G5], dt.float32r, name=f"wl{kk}")
                 for kk in range(KC)]
        wr_sb = [wpool.tile([P, G5], dt.float32r, name=f"wr{kk}")
                 for kk in range(KC)]
        WCH = G5 // 4      # 640-column (5 gate-chunk) load granularity
        for mq in range(4):
            cs_ = slice(mq * WCH, (mq + 1) * WCH)
            for kk in range(KC):
                nc.sync.dma_start(out=wl_sb[kk][:, cs_],
                                  in_=wlt_d.ap()[kk * P:(kk + 1) * P, cs_])
                nc.sync.dma_start(out=wr_sb[kk][:, cs_],
                                  in_=wrt_d.ap()[kk * P:(kk + 1) * P, cs_])

        h_scr = dpool.tile([H, scr_cols], dt.float32r, name="h_scr")
        c_scr = dpool.tile([H, scr_cols], dt.float32, name="c_scr")

        RES_FROM = 2          # outputs of k >= RES_FROM stay SBUF-resident

        for _rep in range(reps):
          h_state = c_state = None          # previous level's resident tiles
          for k in range(DEPTH):
            n_lvl = lvl_n[k]
            S = min(512 if k == 0 else S_MAX, n_lvl)
            W = 2 * S
            resident = k >= RES_FROM
            if resident:
                h_lvl = statp.tile([P, KC * n_lvl], dt.float32r,
                                   tag="state_h", name="h_lvl")
                c_lvl = statp.tile([P, KC * n_lvl], dt.float32,
                                   tag="state_c", name="c_lvl")
            h_prev, c_prev = h_state, c_state
            if k in (4, 5):
                # --- z^T levels: nodes on PSUM partitions, 512 gate columns
                # moving (full-rate fp32r), bias via a K=1 ones-row matmul.
                Sz = n_lvl
                npv = lvl_n[k - 1]
                roles = ("i", "fl", "fr", "cc", "o")
                sigT = {r: sigp.tile([P, H], dt.float32, tag=f"sig_{r}",
                                     name=f"sigT_{r}") for r in roles}
                for q in range(5):
                    psz = psg.tile([P, 512], dt.float32, tag="gate", name="psz")
                    first = True
                    for side in (0, 1):
                        wsb = wl_sb if side == 0 else wr_sb
                        for kk in range(KC):
                            nc.tensor.matmul(
                                psz[:Sz, :],
                                lhsT=h_prev[:, kk * npv + side:
                                            (kk + 1) * npv:2],
                                rhs=wsb[kk][:, q * 512:(q + 1) * 512],
                                start=first, stop=False)
                            first = False
                    nc.tensor.matmul(
                        psz[:Sz, :], lhsT=ones_r[:, :Sz],
                        rhs=biasr_sb[:, q * 512:(q + 1) * 512],
                        start=False, stop=True)
                    for j in range(KC):
                        m = 4 * q + j
                        g, pos = divmod(m, 5)
                        nc.scalar.activation(
                            sigT[roles[pos]][:Sz, g * P:(g + 1) * P],
                            psz[:Sz, j * P:(j + 1) * P],
                            AF.Tanh if pos == 3 else AF.Sigmoid)

                # children c, node-major via PE transposes
                cT_in = []
                for side, tg in ((0, "ta"), (1, "u")):
                    pct = pst.tile([P, H], dt.float32, tag="tr", name="pct")
                    for g in range(KC):
                        nc.tensor.transpose(
                            pct[:Sz, g * P:(g + 1) * P],
                            c_prev[:, g * npv + side:(g + 1) * npv:2],
                            ident[:])
                    ct = tmpp.tile([P, H], dt.float32, tag=tg, name="ct")
                    nc.vector.tensor_copy(ct[:Sz, :], pct[:Sz, :])
                    cT_in.append(ct)

                taT = tmpp.tile([P, H], dt.float32, tag="v", name="taT")
                nc.vector.tensor_mul(taT[:Sz, :], sigT["i"][:Sz, :],
                                     sigT["cc"][:Sz, :])
                u2 = tmpp.tile([P, H], dt.float32, tag="w2", name="u2")
                nc.vector.tensor_mul(u2[:Sz, :], sigT["fl"][:Sz, :],
                                     cT_in[0][:Sz, :])
                v2 = tmpp.tile([P, H], dt.float32, tag="tcq", name="v2")
                nc.vector.tensor_mul(v2[:Sz, :], sigT["fr"][:Sz, :],
                                     cT_in[1][:Sz, :])
                w2T = tmpp.tile([P, H], dt.float32, tag="ta", name="w2T")
                nc.vector.tensor_add(w2T[:Sz, :], taT[:Sz, :], u2[:Sz, :])
                cT = tmpp.tile([P, H], dt.float32, tag="u", name="cT")
                nc.vector.tensor_add(cT[:Sz, :], w2T[:Sz, :], v2[:Sz, :])
                thT = tmpp.tile([P, H], dt.float32, tag="v", name="thT")
                nc.scalar.activation(thT[:Sz, :], cT[:Sz, :], AF.Tanh)
                hT = tmpp.tile([P, H], dt.float32, tag="w2", name="hT")
                nc.vector.tensor_mul(hT[:Sz, :], sigT["o"][:Sz, :],
                                     thT[:Sz, :])

                # back to H-major chunk-major state tiles
                for srcT, dstT in ((hT, h_lvl), (cT, c_lvl)):
                    psb = pst.tile([P, 512], dt.float32, tag="tr", name="psb")
                    for g in range(KC):
                        nc.tensor.transpose(
                            psb[:, g * Sz:(g + 1) * Sz],
                            srcT[:Sz, g * P:(g + 1) * P],
                            ident[:Sz, :Sz])
                    nc.vector.tensor_copy(dstT[:, :], psb[:, :KC * Sz])
            else:
              for it in range(n_lvl // S):
                a = it * S
                col0 = lvl_base[k] + a

                if k == 0:
                    hL = hcin.tile([P, KC * S], dt.float32r, tag="h_in", name="hL")
                    hR = hcin.tile([P, KC * S], dt.float32r, tag="c_in", name="hR")
                    for tt in range(S // 128):
                        t = a // 128 + tt
                        for par, dst in ((0, hL), (1, hR)):
                            stage = leafp.tile([P, H], dt.float32, tag="stage",
                                               name="stage")
                            nc.gpsimd.indirect_dma_start(
                                out=stage[:],
                                out_offset=None,
                                in_=emb_d.ap(),
                                in_offset=IndirectOffsetOnAxis(
                                    ap=idx_sb[:, 2 * t + par:2 * t + par + 1],
                                    axis=0,
                                ),
                            )
                            for c in range(KC):
                                ptr = pst.tile([P, P], dt.float32, tag="tr",
                                               name="ptr")
                                nc.tensor.transpose(
                                    ptr[:], stage[:, c * P:(c + 1) * P],
                                    ident[:])
                                o0 = c * S + tt * 128
                                nc.vector.tensor_copy(dst[:, o0:o0 + 128], ptr[:])

                    def rhs(kk, side, _hL=hL, _hR=hR, _S=S):
                        src = _hL if side == 0 else _hR
                        return src[:, kk * _S:(kk + 1) * _S]

                    cin = None
                elif k <= RES_FROM:
                    # children streamed back from DRAM scratch
                    c0 = lvl_base[k - 1] + 2 * a
                    h_in = hcin.tile([P, KC * W], dt.float32r, tag="h_in",
                                     name="h_in")
                    c_in = hcin.tile([P, KC * W], dt.float32, tag="c_in",
                                     name="c_in")
                    for w0 in range(0, W, 256):
                        hw_ = min(256, W - w0)
                        nc.sync.dma_start(
                            out=s3(h_in)[:, :, w0:w0 + hw_],
                            in_=d3(h_scr[:, c0:c0 + W])[:, :, w0:w0 + hw_])
                        nc.sync.dma_start(
                            out=s3(c_in)[:, :, w0:w0 + hw_],
                            in_=d3(c_scr[:, c0:c0 + W])[:, :, w0:w0 + hw_])

                    def rhs(kk, side, _h=h_in, _W=W):
                        b0 = kk * _W
                        return _h[:, b0 + side:b0 + _W:2]

                    def cin(g, side, _c=c_in, _W=W):
                        b0 = g * _W
                        return _c[:, b0 + side:b0 + _W:2]
                else:
                    # children live in the previous level's resident tiles
                    n_prev = 2 * n_lvl

                    def rhs(kk, side, _h=h_prev, _np=n_prev, _a=2 * a, _W=W):
                        b0 = kk * _np + _a
                        return _h[:, b0 + side:b0 + _W:2]

                    def cin(g, side, _c=c_prev, _np=n_prev, _a=2 * a, _W=W):
                        b0 = g * _np + _a
                        return _c[:, b0 + side:b0 + _W:2]

                if resident:
                    h_out = h_lvl[:, :]
                    c_out = c_lvl[:, :]
                    ho_sl = lambda g, _a=a, _n=n_lvl, _S=S: \
                        h_out[:, g * _n + _a:g * _n + _a + _S]
                    co_sl = lambda g, _a=a, _n=n_lvl, _S=S: \
                        c_out[:, g * _n + _a:g * _n + _a + _S]
                else:
                    h_out = outp.tile([P, KC * S], dt.float32r, tag="h_out",
                                      name="h_out")
                    c_out = outp.tile([P, KC * S], dt.float32, tag="c_out",
                                      name="c_out")
                    ho_sl = lambda g, _S=S: h_out[:, g * _S:(g + 1) * _S]
                    co_sl = lambda g, _S=S: c_out[:, g * _S:(g + 1) * _S]

                # fp32r matmuls drop to 1/4 rate below 256 moving columns;
                # at S=128 recover full rate by streaming the rhs twice via
                # a 0-stride AP dim (psum columns S..2S are discarded dups)
                pad = 2 if (S == 128 and k > 0) else 1

                for g in range(KC):
                    sigs = {}
                    # weights are host-permuted g-major: the 5 gate chunks
                    # for H-chunk g sit at m = 5g..5g+4 (i, fl, fr, cc, o)
                    for role, m in (("i", 5 * g), ("fl", 5 * g + 1),
                                    ("fr", 5 * g + 2), ("cc", 5 * g + 3),
                                    ("o", 5 * g + 4)):
                        ps = psg.tile([P, S * pad], dt.float32, tag="gate",
                                      name="ps")
                        first = True
                        for side in (0, 1):
                            wsb = wl_sb if side == 0 else wr_sb
                            for kk in range(KC):
                                r_ap = rhs(kk, side)
                                if pad == 2:
                                    r_ap = bass_mod.AP(
                                        r_ap.tensor, r_ap.offset,
                                        [list(r_ap.ap[0]), [0, 2],
                                         list(r_ap.ap[-1])])
                                nc.tensor.matmul(
                                    ps[:],
                                    lhsT=wsb[kk][:, m * P:(m + 1) * P],
                                    rhs=r_ap,
                                    start=first,
                                    stop=(side == 1 and kk == KC - 1),
                                )
                                first = False
                        sg = sigp.tile([P, S], dt.float32, tag=f"sig_{role}",
                                       name=f"sig_{role}")
                        nc.scalar.activation(
                            sg[:], ps[:, :S],
                            AF.Tanh if role == "cc" else AF.Sigmoid,
                            bias=bias_sb[:, m:m + 1],
                        )
                        sigs[role] = sg

                    cq = co_sl(g)
                    ta = tmpp.tile([P, S], dt.float32, tag="ta", name="ta")
                    nc.vector.tensor_mul(ta[:], sigs["i"][:], sigs["cc"][:])
                    if k == 0:
                        u = tmpp.tile([P, S], dt.float32, tag="u", name="u")
                        nc.vector.tensor_add(u[:], sigs["fl"][:], sigs["fr"][:])
                        nc.vector.scalar_tensor_tensor(
                            out=cq, in0=u[:], scalar=0.5, in1=ta[:],
                            op0=OP.mult, op1=OP.add)
                    else:
                        u = tmpp.tile([P, S], dt.float32, tag="u", name="u")
                        nc.vector.tensor_mul(u[:], sigs["fl"][:], cin(g, 0))
                        v = tmpp.tile([P, S], dt.float32, tag="v", name="v")
                        nc.vector.tensor_mul(v[:], sigs["fr"][:], cin(g, 1))
                        w2 = tmpp.tile([P, S], dt.float32, tag="w2", name="w2")
                        nc.vector.tensor_add(w2[:], ta[:], u[:])
                        nc.vector.tensor_add(cq, w2[:], v[:])
                    tcq = tmpp.tile([P, S], dt.float32, tag="tcq", name="tcq")
                    nc.scalar.activation(tcq[:], cq, AF.Tanh)
                    nc.vector.tensor_mul(ho_sl(g), sigs["o"][:], tcq[:])

                if not resident:
                    # per-tile stores of h (output + scratch) and c (scratch)
                    nc.sync.dma_start(out=d3(hs_d.ap()[:, col0:col0 + S]),
                                      in_=s3(h_out))
                    nc.sync.dma_start(out=d3(h_scr[:, col0:col0 + S]),
                                      in_=s3(h_out))
                    nc.sync.dma_start(out=d3(c_scr[:, col0:col0 + S]),
                                      in_=s3(c_out))

            if resident:
                lb = lvl_base[k]
                nc.sync.dma_start(
                    out=d3(hs_d.ap()[:, lb:lb + n_lvl]), in_=s3(h_lvl[:, :]))
                if k == DEPTH - 1:
                    nc.sync.dma_start(out=d3(croot_d.ap()),
                                      in_=s3(c_lvl[:, :]))
                h_state, c_state = h_lvl, c_lvl

    nc.compile()
    _PROG_CACHE[(nt, reps)] = nc
    return nc


# ---------------------------------------------------------------- host side
def _prep_inputs(word_idx, emb, Wl, Wr, b, ncores=NCORES, nt=NT):
    """Build per-core input maps."""
    wi = np.asarray(word_idx).astype(np.int32)          # (B, L)
    # idx[core][p, 2*t+par] = word_idx[core*nt + t, 2*p + par]
    idx = wi.reshape(ncores, nt, 128, 2).transpose(0, 2, 1, 3) \
        .reshape(ncores, 128, 2 * nt)
    idx = np.ascontiguousarray(idx)

    embf = np.ascontiguousarray(np.asarray(emb, dtype=np.float32))

    # permute gate rows g-major: new chunk 5g+pos = old chunk r*KC+g where
    # pos->(i,fl,fr,cc,o) maps to old gate r in (0,1,2,4,3)
    perm = np.empty(MC, dtype=np.int64)
    for g in range(KC):
        for pos, r in enumerate((0, 1, 2, 4, 3)):
            perm[5 * g + pos] = r * KC + g
    row_perm = (perm[:, None] * P + np.arange(P)[None, :]).reshape(-1)

    wlt = np.ascontiguousarray(Wl[row_perm].T.astype(np.float32))   # (H, 5H)
    wrt = np.ascontiguousarray(Wr[row_perm].T.astype(np.float32))
    bias_t = np.ascontiguousarray(
        b[row_perm].astype(np.float32).reshape(MC, P).T)            # (128, 20)

    bias_r = np.ascontiguousarray(b[row_perm].astype(np.float32)
                                  .reshape(1, G5))
    return [
        {"idx": idx[c], "emb": embf, "wlt": wlt, "wrt": wrt, "bias": bias_t,
         "bias_r": bias_r}
        for c in range(ncores)
    ]


def _assemble(results, ncores=NCORES, nt=NT):
    """Gather per-core (hs, c_root) into the reference output pytree."""
    lvl_n, lvl_base = _lvl_meta(nt)
    H_int = np.empty((ncores * nt, L - 1, H), dtype=np.float32)
    fh = np.empty((1, ncores * nt, H), dtype=np.float32)
    fc = np.empty((1, ncores * nt, H), dtype=np.float32)
    for c in range(ncores):
        hs = np.asarray(results[c]["hs"])            # (512, nt*255)
        croot = np.asarray(results[c]["c_root"])     # (512, nt)
        for k in range(DEPTH):
            nk = 128 >> k
            blk = hs[:, lvl_base[k]:lvl_base[k + 1]].reshape(H, nt, nk)
            H_int[c * nt:(c + 1) * nt, OFF[k + 1]:OFF[k + 1] + nk, :] = \
                blk.transpose(1, 2, 0)
        fh[0, c * nt:(c + 1) * nt, :] = \
            hs[:, lvl_base[DEPTH - 1]:lvl_base[DEPTH]].T
        fc[0, c * nt:(c + 1) * nt, :] = croot.T
    out = np.concatenate([H_int[:, POST[:1]], H_int[:, POST]], axis=1)
    return out, (fh, fc)


def kernel(word_idx, emb, U_i_l, U_i_r, b_i, U_fl_l, U_fl_r, b_fl,
           U_fr_l, U_fr_r, b_fr, U_o_l, U_o_r, b_o, U_c_l, U_c_r, b_c):
    from concourse.bass_utils import run_bass_kernel_spmd

    Wl = np.concatenate([np.asarray(x, np.float32)
                         for x in (U_i_l, U_fl_l, U_fr_l, U_o_l, U_c_l)], axis=0)
    Wr = np.concatenate([np.asarray(x, np.float32)
                         for x in (U_i_r, U_fl_r, U_fr_r, U_o_r, U_c_r)], axis=0)
    b = np.concatenate([np.asarray(x, np.float32)
                        for x in (b_i, b_fl, b_fr, b_o, b_c)], axis=0)

    in_maps = _prep_inputs(word_idx, emb, Wl, Wr, b)
    nc = _build_program(NT)
    res = run_bass_kernel_spmd(nc, in_maps, list(range(NCORES)))
    return _assemble(res.results)
